# revision 11
# baseline (speedup 1.0000x reference)
"""Trainium2 Bass kernel for a GRU decoder with Luong attention.

Problem (hardcoded shapes): B=32, S=64, T=64, H=512, V=32000.
  out = log_softmax(decoder(inputs)) with shape [B, T, V] fp32.

Sharding: data-parallel over batch. Each of the 8 cores processes 4 batch
rows end-to-end. No collectives.

Design notes:
- GRU recurrence runs fully in transposed layout: tiles [128, 16] with
  partition = h-dim within a 128-chunk, col = q*4 + b (q = h chunk, b =
  local batch row). All gates via Tanh only (sigmoid folded via
  0.5*(1+tanh(x/2))), so the activation table never switches between
  Tanh and Exp (same table).
- Output projection in fp8 (e4m3) with DoubleRow perf mode; weights are
  pre-scaled by WS=8 and the scale is undone in the exp / subtract ops.
- log-softmax in two passes over vocab chunks: pass A computes
  exp-accumulate row sums straight out of PSUM; pass B recomputes the
  matmuls and does (psum/WS - lse) -> bf16 -> DRAM. No SBUF staging of
  the logits.
- Output rows per core are r = t*4 + b_local; stored bf16, host converts
  to f32.
"""

from contextlib import ExitStack

import numpy as np
import ml_dtypes

import concourse.bacc as bacc
import concourse.bass as bass
import concourse.mybir as mybir
import concourse.tile as tile
from concourse.masks import make_identity

F32 = mybir.dt.float32
BF16 = mybir.dt.bfloat16
FP8 = mybir.dt.float8e4
I32 = mybir.dt.int32
AF = mybir.ActivationFunctionType
ALU = mybir.AluOpType
AX = mybir.AxisListType
F32R = mybir.dt.float32r
DR = mybir.MatmulPerfMode.DoubleRow


def rr(ap):
    return ap.bitcast(F32R)


B, S, T, H, V = 32, 64, 64, 512, 32000
NC = 8
BL = B // NC          # 4 local batch rows
R = T * BL            # 256 local output rows, r = t*BL + b
NEG = -1e30
WS = 8.0              # fp8 weight scale for the output projection
VC = 512              # vocab half-chunk (one PSUM bank)
NPAIR = 32            # pair-chunks of 2*VC; last pair is 256 wide
PAIRW = [2 * VC] * (NPAIR - 1) + [256]   # psum cols per pair
PAIROFF = [2 * VC * i for i in range(NPAIR)]
# number of A-phase (m=0) pair chunks interleaved into the GRU t-loop
OVERLAP_T0 = 39       # first t step that emits an overlapped A chunk


def build_program():
    nc = bacc.Bacc(None, target_bir_lowering=False, debug=False)

    # ---- DRAM parameters (per-core slices prepared on host) ----
    emb_d = nc.declare_dram_parameter("emb", [V, H], F32, isOutput=False)
    ids_d = nc.declare_dram_parameter("ids", [2, 128, 1], I32, isOutput=False)
    h0T_d = nc.declare_dram_parameter("h0T", [128, 16], F32, isOutput=False)
    encT_d = nc.declare_dram_parameter("encT", [H, BL * S], F32, isOutput=False)
    encS_d = nc.declare_dram_parameter("encS", [S, BL * H], F32, isOutput=False)
    maskb_d = nc.declare_dram_parameter("maskb", [1, BL * S], F32, isOutput=False)
    actmT_d = nc.declare_dram_parameter("actmT", [128, T * 16], F32, isOutput=False)
    wihT_d = nc.declare_dram_parameter("wihT", [H, 3 * H], F32, isOutput=False)
    whhT_d = nc.declare_dram_parameter("whhT", [H, 3 * H], F32, isOutput=False)
    brow_d = nc.declare_dram_parameter("brow", [1, 3 * H], F32, isOutput=False)
    bhhn4_d = nc.declare_dram_parameter("bhhn4", [4, 128], F32, isOutput=False)
    sel4_d = nc.declare_dram_parameter("sel4", [4, 16], F32, isOutput=False)
    wccT_d = nc.declare_dram_parameter("wccT", [2 * H, H], F32, isOutput=False)
    bcc_d = nc.declare_dram_parameter("bcc", [128, 4], F32, isOutput=False)
    # fp8 weight bytes disguised as int32 for the PJRT interface
    wo8_d = nc.declare_dram_parameter("wo8", [2, 128, V // 2], I32, isOutput=False)
    ones_d = nc.declare_dram_parameter("onesd", [1, 256], F32, isOutput=False)
    out_d = nc.declare_dram_parameter("out", [R, V], BF16, isOutput=True)

    with tile.TileContext(nc) as tc, ExitStack() as stk:
        constp = stk.enter_context(tc.tile_pool(name="const", bufs=1))
        histp = stk.enter_context(tc.tile_pool(name="hist", bufs=1))
        hotp = stk.enter_context(tc.tile_pool(name="hot", bufs=1))
        wop = stk.enter_context(tc.tile_pool(name="wo", bufs=6))
        sump = stk.enter_context(tc.tile_pool(name="sums", bufs=1))
        dmp = stk.enter_context(tc.tile_pool(name="dump", bufs=2))
        ostp = stk.enter_context(tc.tile_pool(name="ost", bufs=3))
        ps_A = stk.enter_context(tc.tile_pool(name="ps_A", bufs=2, space="PSUM"))

        # ---- constants / small tiles ----
        ident = constp.tile([128, 128], F32, tag="ident")
        make_identity(nc, ident[:])
        identr = constp.tile([128, 128], F32, tag="identr")
        nc.vector.tensor_copy(rr(identr[:]), ident[:])
        ones256 = constp.tile([1, 256], F32, tag="ones256")
        nc.sync.dma_start(rr(ones256[:]), rr(ones_d[:]))
        maskb = constp.tile([1, BL * S], F32, tag="maskb")
        nc.sync.dma_start(rr(maskb[:]), rr(maskb_d[:]))
        actmT = constp.tile([128, T * 16], F32, tag="actmT")
        nc.sync.dma_start(actmT[:], actmT_d[:])
        bcc = constp.tile([128, 4], F32, tag="bcc")
        nc.sync.dma_start(bcc[:], bcc_d[:])
        bhhn4 = constp.tile([4, 128], F32, tag="bhhn4")
        nc.sync.dma_start(rr(bhhn4[:]), rr(bhhn4_d[:]))
        sel4 = constp.tile([4, 16], F32, tag="sel4")
        nc.sync.dma_start(rr(sel4[:]), rr(sel4_d[:]))
        brow = constp.tile([1, 3 * H], F32, tag="brow")
        nc.sync.dma_start(rr(brow[:]), rr(brow_d[:]))

        # history tiles: col = t*16 + q*4 + b
        hnewT = histp.tile([128, T * 16], F32, tag="hnewT")
        ctxT = histp.tile([128, T * 16], F32, tag="ctxT")
        # hot in fp8, DoubleRow layout: hotA covers h-chunks (0,1),
        # hotB (2,3); col = k*R + r
        hotA = hotp.tile([128, 2 * R], FP8, tag="hotA")
        hotB = hotp.tile([128, 2 * R], FP8, tag="hotB")
        sets = [sump.tile([128, NPAIR], F32, tag=f"sets{m}", name=f"sets{m}")
                for m in range(2)]
        lsen = [sump.tile([128, 1], F32, tag=f"lsen{m}", name=f"lsen{m}")
                for m in range(2)]

        # ---------- P5 chunk emitters ----------
        def load_w(jp, eng):
            """Load the two fp8 weight pair-tiles for pair-chunk jp."""
            w = PAIRW[jp]
            tiles = []
            for p in range(2):
                wt = wop.tile([128, 2048], FP8, tag="wch", name=f"w{p}_{jp}")
                o4 = PAIROFF[jp] // 2
                eng.dma_start(
                    wt[:, 0:2 * w].bitcast(I32), wo8_d[p][:, o4:o4 + w // 2]
                )
                tiles.append(wt)
            return tiles

        def emit_mms(ps, m, jp, wt, hots):
            """Accumulate logits for rows m*128.. of pair-chunk jp into ps."""
            w = PAIRW[jp]
            nhalf = 2 if w == 2 * VC else 1
            for hf in range(nhalf):
                wh = min(VC, w - hf * VC)
                dst = ps[:, hf * VC:hf * VC + wh]
                for p in range(2):
                    rhs = wt[p][:, 0:2 * w].rearrange("p (k n) -> p k n", k=2)[
                        :, :, hf * VC:hf * VC + wh
                    ]
                    lhsT = hots[p][:].rearrange("p (k r) -> p k r", k=2)[
                        :, :, m * 128:(m + 1) * 128
                    ]
                    nc.tensor.matmul(dst, lhsT, rhs, start=(p == 0),
                                     stop=(p == 1), perf_mode=DR)

        def emit_A(m, jp, dma_eng):
            """Pass A: matmuls + exp-accumulate row sums for pair jp."""
            wt = load_w(jp, dma_eng)
            w = PAIRW[jp]
            ps = ps_A.tile([128, 2 * VC], F32, tag="A", name=f"psA{m}_{jp}")
            emit_mms(ps, m, jp, wt, (hotA, hotB))
            dump = dmp.tile([128, 2 * VC], BF16, tag="dump", name=f"dm{m}_{jp}")
            nc.scalar.activation(
                dump[:, 0:w], ps[:, 0:w], AF.Exp, scale=float(1.0 / WS),
                accum_out=sets[m][:, jp:jp + 1],
            )

        def emit_B(ps_pool, m, jp, dma_eng, st_eng, sub_eng, wt=None):
            """Pass B: recompute, subtract lse, store bf16."""
            if wt is None:
                wt = load_w(jp, dma_eng)
            w = PAIRW[jp]
            ps = ps_pool.tile([128, 2 * VC], F32, tag="B", name=f"psB{m}_{jp}")
            emit_mms(ps, m, jp, wt, (hotA, hotB))
            ost = ostp.tile([128, 2 * VC], BF16, tag="ost", name=f"os{m}_{jp}")
            if sub_eng is None:
                nc.scalar.activation(
                    ost[:, 0:w], ps[:, 0:w], AF.Identity,
                    scale=float(1.0 / WS), bias=lsen[m][:, 0:1],
                )
            else:
                sub_eng.tensor_scalar(
                    ost[:, 0:w], ps[:, 0:w], float(1.0 / WS),
                    lsen[m][:, 0:1], ALU.mult, ALU.add,
                )
            st_eng.dma_start(
                out_d[m * 128:(m + 1) * 128, PAIROFF[jp]:PAIROFF[jp] + w],
                ost[:, 0:w],
            )

        def emit_lse(m):
            stot = sump.tile([128, 1], F32, tag=f"st{m}", name=f"stot{m}")
            nc.vector.tensor_reduce(stot[:], sets[m][:], AX.X, ALU.add)
            rec = sump.tile([128, 1], F32, tag=f"rc{m}", name=f"rec{m}")
            nc.vector.reciprocal(rec[:], stot[:])
            # ln(1/sum) = -lse  (as the Identity/tensor_scalar additive bias)
            nc.scalar.activation(lsen[m][:], rec[:], AF.Ln)

        with ExitStack() as stk1:
            wp = stk1.enter_context(tc.tile_pool(name="weights", bufs=1))
            xsp = stk1.enter_context(tc.tile_pool(name="xs", bufs=2))
            xstp = stk1.enter_context(tc.tile_pool(name="xsT", bufs=1))
            gxp = stk1.enter_context(tc.tile_pool(name="gx", bufs=1))
            hTp = stk1.enter_context(tc.tile_pool(name="hT", bufs=2))
            gp = stk1.enter_context(tc.tile_pool(name="gates", bufs=2))
            attp = stk1.enter_context(tc.tile_pool(name="att", bufs=2))
            ps_g = stk1.enter_context(tc.tile_pool(name="ps_g", bufs=2, space="PSUM"))
            ps_a = stk1.enter_context(tc.tile_pool(name="ps_a", bufs=2, space="PSUM"))

            # ---- weight loads (recurrence path) ----
            wih, whh = [], []
            for q in range(4):
                wt_ = wp.tile([128, 3 * H], F32, tag=f"wih{q}")
                nc.sync.dma_start(rr(wt_[:]), rr(wihT_d[q * 128:(q + 1) * 128, :]))
                wih.append(wt_)
            for q in range(4):
                ht_ = wp.tile([128, 3 * H], F32, tag=f"whh{q}")
                nc.gpsimd.dma_start(rr(ht_[:]), rr(whhT_d[q * 128:(q + 1) * 128, :]))
                whh.append(ht_)
            wcc = []
            for kt in range(8):
                w_ = wp.tile([128, H], F32, tag=f"wcc{kt}")
                nc.gpsimd.dma_start(rr(w_[:]), rr(wccT_d[kt * 128:(kt + 1) * 128, :]))
                wcc.append(w_)
            encT = []
            for q in range(4):
                e_ = wp.tile([128, BL * S], F32, tag=f"encT{q}")
                nc.gpsimd.dma_start(rr(e_[:]), rr(encT_d[q * 128:(q + 1) * 128, :]))
                encT.append(e_)
            encS = wp.tile([S, BL * H], F32, tag="encS")
            nc.gpsimd.dma_start(rr(encS[:]), rr(encS_d[:]))

            # ---- P1: embedding gather + gxT = (x @ W_ih.T + b).T ----
            xsT = [xstp.tile([128, 256], F32, tag=f"xsT{q}", name=f"xsT{q}")
                   for q in range(4)]
            for m in range(2):
                ids_t = xsp.tile([128, 1], I32, tag="ids")
                nc.sync.dma_start(ids_t[:], ids_d[m])
                xs_t = xsp.tile([128, H], F32, tag="xs")
                nc.gpsimd.indirect_dma_start(
                    out=xs_t[:],
                    out_offset=None,
                    in_=emb_d[:],
                    in_offset=bass.IndirectOffsetOnAxis(ap=ids_t[:, 0:1], axis=0),
                )
                for q in range(4):
                    tp = ps_a.tile([128, 128], F32, tag="A", name=f"tp{m}_{q}")
                    nc.tensor.transpose(tp[:], xs_t[:, q * 128:(q + 1) * 128], ident[:])
                    nc.vector.tensor_copy(rr(xsT[q][:, m * 128:(m + 1) * 128]), tp[:])

            gxT_rz = gxp.tile([128, T * 32], F32, tag="gxrz")
            gxT_n = gxp.tile([128, T * 16], F32, tag="gxn")
            vrz = gxT_rz[:].rearrange("p (t j x) -> p t j x", j=2, x=16)
            vn = gxT_n[:].rearrange("p (t x) -> p t x", x=16)
            for j in range(3):
                for ms in range(4):
                    col = j * 512 + ms * 128
                    gps = ps_a.tile([128, 256], F32, tag="A", name=f"gx{j}_{ms}")
                    for q in range(4):
                        nc.tensor.matmul(
                            gps[:], rr(wih[q][:, col:col + 128]), rr(xsT[q][:]),
                            start=(q == 0), stop=False,
                        )
                    nc.tensor.matmul(
                        gps[:], rr(brow[0:1, col:col + 128]), rr(ones256[0:1, :]),
                        start=False, stop=True,
                    )
                    src = gps[:].rearrange("p (t b) -> p t b", b=4)
                    if j < 2:
                        dst = vrz[:, :, j, ms * 4:(ms + 1) * 4]
                    else:
                        dst = vn[:, :, ms * 4:(ms + 1) * 4]
                    nc.vector.tensor_copy(dst.bitcast(F32R), src)

            # ---- P2: GRU recurrence (transposed layout) + attention ----
            hT = hTp.tile([128, 16], F32, tag="hT", name="hT_init")
            nc.sync.dma_start(rr(hT[:]), rr(h0T_d[:]))

            def emit_attention(blk, b):
                """Scores+softmax+context for (block blk, local batch b)."""
                c0 = blk * 256
                sc = ps_a.tile([16, S], F32, tag="A", name=f"sc{blk}_{b}")
                for q in range(4):
                    nc.tensor.matmul(
                        sc[:],
                        rr(hnewT[:, c0 + q * 4 + b:c0 + 256:16]),
                        rr(encT[q][:, b * S:(b + 1) * S]),
                        start=(q == 0), stop=False,
                    )
                nc.tensor.matmul(
                    sc[:], rr(ones256[0:1, 0:16]), rr(maskb[0:1, b * S:(b + 1) * S]),
                    start=False, stop=True,
                )
                nmax = attp.tile([16, 1], F32, tag="nmax", name=f"nm{blk}_{b}")
                nc.vector.tensor_reduce(nmax[:], sc[:], AX.X, ALU.max, negate=True)
                se = attp.tile([16, 1], F32, tag="se", name=f"se{blk}_{b}")
                al = attp.tile([16, S], F32, tag="al", name=f"al{blk}_{b}")
                nc.scalar.activation(al[:], sc[:], AF.Exp, bias=nmax[:, 0:1],
                                     accum_out=se[:, 0:1])
                rec = attp.tile([16, 1], F32, tag="rec", name=f"rc{blk}_{b}")
                nc.vector.reciprocal(rec[:], se[:])
                aln = attp.tile([16, S], F32, tag="aln", name=f"an{blk}_{b}")
                nc.vector.tensor_scalar_mul(aln[:], al[:], rec[:, 0:1])
                alT_ps = ps_a.tile([S, 16], F32, tag="A", name=f"tpa{blk}_{b}")
                nc.tensor.transpose(alT_ps[:], aln[:], ident[0:16, 0:16])
                alT = attp.tile([S, 16], F32, tag="alT", name=f"at{blk}_{b}")
                nc.vector.tensor_copy(rr(alT[:]), alT_ps[:])
                cx = ps_a.tile([128, 64], F32, tag="A", name=f"cx{blk}_{b}")
                for q in range(4):
                    nc.tensor.matmul(
                        cx[:, q * 16:(q + 1) * 16],
                        rr(encS[0:S, b * H + q * 128:b * H + (q + 1) * 128]),
                        rr(alT[:]),
                        start=(q == 0), stop=(q == 3),
                    )
                csrc = cx[:].rearrange("p (q t) -> p q t", q=4)
                cdst = ctxT[:].rearrange("p (t q x) -> p q t x", q=4, x=4)[
                    :, :, blk * 16:(blk + 1) * 16, b
                ]
                nc.vector.tensor_copy(cdst.bitcast(F32R), csrc)

            def emit_hot(blk):
                for mh in range(4):
                    hps = ps_a.tile([128, 64], F32, tag="A", name=f"hp{blk}_{mh}")
                    for kt in range(8):
                        srcT = ctxT if kt < 4 else hnewT
                        q = kt % 4
                        rhs = srcT[:].rearrange("p (t x) -> p t x", x=16)[
                            :, blk * 16:(blk + 1) * 16, q * 4:(q + 1) * 4
                        ]
                        nc.tensor.matmul(
                            hps[:], rr(wcc[kt][:, mh * 128:(mh + 1) * 128]), rr(rhs),
                            start=(kt == 0), stop=(kt == 7),
                        )
                    dsttile = hotA if mh < 2 else hotB
                    k = mh % 2
                    nc.scalar.activation(
                        dsttile[:, k * R + blk * 64:k * R + (blk + 1) * 64],
                        hps[:], AF.Tanh, bias=bcc[:, mh:mh + 1],
                    )

            ovl = 0   # next overlapped pass-A (m=0) pair chunk
            for t in range(T):
                ps = ps_g.tile([128, 512], F32, tag="G", name=f"ps{t}")
                # r,z gates -> cols 0:32 ; n gate -> cols 32:48.
                # One PSUM bank: exactly one start (first mm) and one stop
                # (last mm); intermediate slices accumulate into the
                # pending-zero region.
                first = True
                for j in range(2):
                    for ms in range(4):
                        col = j * 512 + ms * 128
                        dst = ps[:, j * 16 + ms * 4:j * 16 + (ms + 1) * 4]
                        for q in range(4):
                            nc.tensor.matmul(
                                dst, rr(whh[q][:, col:col + 128]),
                                rr(hT[:, q * 4:(q + 1) * 4]),
                                start=first, stop=False,
                            )
                            first = False
                nc.tensor.matmul(
                    ps[:, 0:32], rr(identr[:]), rr(gxT_rz[:, t * 32:(t + 1) * 32]),
                    start=False, stop=False,
                )
                for ms in range(4):
                    col = 2 * 512 + ms * 128
                    dst = ps[:, 32 + ms * 4:32 + (ms + 1) * 4]
                    for q in range(4):
                        nc.tensor.matmul(
                            dst, rr(whh[q][:, col:col + 128]),
                            rr(hT[:, q * 4:(q + 1) * 4]),
                            start=False, stop=False,
                        )
                nc.tensor.matmul(
                    ps[:, 32:48], rr(bhhn4[:]), rr(sel4[:]),
                    start=False, stop=True,
                )

                th = gp.tile([128, 32], F32, tag="th", name=f"th{t}")
                nc.scalar.activation(th[:], ps[:, 0:32], AF.Tanh, scale=0.5)
                ghn = gp.tile([128, 16], F32, tag="ghn", name=f"ghn{t}")
                nc.vector.tensor_copy(rr(ghn[:]), ps[:, 32:48])
                thp = gp.tile([128, 16], F32, tag="thp", name=f"thp{t}")
                nc.vector.tensor_scalar_add(thp[:], th[:, 0:16], 1.0)
                a1 = gp.tile([128, 16], F32, tag="a1", name=f"a1{t}")
                nc.vector.tensor_tensor(a1[:], thp[:], ghn[:], ALU.mult)
                a2 = gp.tile([128, 16], F32, tag="a2", name=f"a2{t}")
                nc.vector.tensor_tensor(a2[:], a1[:], gxT_n[:, t * 16:(t + 1) * 16],
                                        ALU.add)
                n_ = gp.tile([128, 16], F32, tag="n", name=f"n{t}")
                nc.scalar.activation(n_[:], a2[:], AF.Tanh, scale=0.5)
                # g1 = 0.5*(1 - th_z) on Pool (parallel with the n-branch)
                g1 = gp.tile([128, 16], F32, tag="g1", name=f"g1{t}")
                nc.gpsimd.tensor_scalar(g1[:], th[:, 16:32], -0.5, 0.5,
                                        ALU.mult, ALU.add)
                g_ = gp.tile([128, 16], F32, tag="g", name=f"g{t}")
                nc.gpsimd.tensor_tensor(g_[:], g1[:], actmT[:, t * 16:(t + 1) * 16],
                                        ALU.mult)
                e_ = gp.tile([128, 16], F32, tag="e", name=f"e{t}")
                nc.vector.tensor_tensor(e_[:], n_[:], hT[:], ALU.subtract)
                u_ = gp.tile([128, 16], F32, tag="u", name=f"u{t}")
                nc.vector.tensor_tensor(u_[:], g_[:], e_[:], ALU.mult)
                hT2 = hTp.tile([128, 16], F32, tag="hT", name=f"hT{t}")
                nc.vector.tensor_tensor(rr(hT2[:]), hT[:], u_[:], ALU.add)
                # hnewT (unmasked h_new) off the critical path, on Pool
                w1 = gp.tile([128, 16], F32, tag="w1", name=f"w1{t}")
                nc.gpsimd.tensor_tensor(w1[:], g1[:], e_[:], ALU.mult)
                nc.gpsimd.tensor_tensor(rr(hnewT[:, t * 16:(t + 1) * 16]), hT[:],
                                        w1[:], ALU.add)
                hT = hT2

                # attention for block blk is spread over steps of block blk+1
                if t >= 16 and t % 16 < 4:
                    emit_attention(t // 16 - 1, t % 16)
                elif t >= 16 and t % 16 == 4:
                    emit_hot(t // 16 - 1)
                # overlapped pass-A chunks for rows m=0 (ready after block 1)
                if t >= OVERLAP_T0 and ovl < NPAIR:
                    emit_A(0, ovl, nc.sync)
                    ovl += 1

            for b in range(4):
                emit_attention(3, b)
            emit_hot(3)

        # ---- P5 (post-loop): remaining pass A, then pass B ----
        with tc.tile_pool(name="ps_B", bufs=2, space="PSUM") as ps_B:
            for jp in range(ovl, NPAIR):
                emit_A(0, jp, nc.sync if jp % 2 == 0 else nc.gpsimd)
            emit_lse(0)
            # A(m=1) and B(m=0) share weight tiles per pair-chunk
            for jp in range(NPAIR):
                wt = load_w(jp, nc.sync)
                w = PAIRW[jp]
                ps = ps_A.tile([128, 2 * VC], F32, tag="A", name=f"psA1_{jp}")
                emit_mms(ps, 1, jp, wt, (hotA, hotB))
                dump = dmp.tile([128, 2 * VC], BF16, tag="dump", name=f"dmA1_{jp}")
                nc.scalar.activation(
                    dump[:, 0:w], ps[:, 0:w], AF.Exp, scale=float(1.0 / WS),
                    accum_out=sets[1][:, jp:jp + 1],
                )
                emit_B(ps_B, 0, jp, None, nc.gpsimd, nc.vector, wt=wt)
            emit_lse(1)
            for jp in range(NPAIR):
                emit_B(ps_B, 1, jp,
                       nc.sync if jp % 2 == 0 else nc.gpsimd,
                       nc.gpsimd if jp % 2 == 0 else nc.sync,
                       nc.vector if jp % 2 == 0 else None)

    nc.compile()
    return nc


_NC_CACHE = None


def _get_program():
    global _NC_CACHE
    if _NC_CACHE is None:
        _NC_CACHE = build_program()
    return _NC_CACHE


def make_core_inputs(all_encoder_hidden_states, initial_decoder_hidden_state,
                     encoder_output_mask, target_input, fra_length, embedding,
                     W_ih, W_hh, b_ih, b_hh, W_cc, b_cc, W_out, b_out):
    """Build the per-core input maps (host-side sharding/layout only)."""
    f8 = ml_dtypes.float8_e4m3
    enc = np.ascontiguousarray(np.asarray(all_encoder_hidden_states, np.float32))
    h0 = np.asarray(initial_decoder_hidden_state, np.float32)[0]
    mask = np.asarray(encoder_output_mask)
    tgt = np.asarray(target_input).astype(np.int64)
    fra = np.asarray(fra_length).astype(np.int64)
    emb = np.ascontiguousarray(np.asarray(embedding, np.float32))
    W_ih = np.asarray(W_ih, np.float32)
    W_hh = np.asarray(W_hh, np.float32)
    b_ih = np.asarray(b_ih, np.float32)
    b_hh = np.asarray(b_hh, np.float32)

    wih_mod = W_ih.copy()
    wih_mod[2 * H:3 * H, :] *= 2.0          # n-gate pre-scaled by 2
    wihT = np.ascontiguousarray(wih_mod.T)
    whhT = np.ascontiguousarray(W_hh.T)
    brow = np.concatenate([
        (b_ih[:2 * H] + b_hh[:2 * H]),      # r,z: both biases, fold into gx
        2.0 * b_ih[2 * H:],                 # n: only b_ih (scaled)
    ])[None, :].astype(np.float32)
    bhhn4 = np.ascontiguousarray(b_hh[2 * H:].reshape(4, 128))
    sel4 = np.ascontiguousarray(np.repeat(np.eye(4, dtype=np.float32), 4, axis=1))
    wccT = np.ascontiguousarray(np.asarray(W_cc, np.float32).T)
    bcc4 = np.ascontiguousarray(np.asarray(b_cc, np.float32).reshape(4, 128).T)

    # fp8 output weights, DoubleRow pair layout
    w8 = (np.asarray(W_out, np.float32).T * WS).astype(f8)   # [H, V]
    wt4 = w8.reshape(4, 128, V)
    wo8 = np.empty((2, 128, 2 * V), f8)
    for p in range(2):
        blocks = []
        for jp in range(NPAIR):
            sl = slice(PAIROFF[jp], PAIROFF[jp] + PAIRW[jp])
            blocks.append(np.concatenate([wt4[2 * p][:, sl], wt4[2 * p + 1][:, sl]],
                                         axis=1))
        wo8[p] = np.concatenate(blocks, axis=1)
    wo8i = np.ascontiguousarray(wo8).view(np.uint8).astype(np.uint8).view(np.int32).reshape(2, 128, V // 2)

    in_maps = []
    for c in range(NC):
        bs = slice(c * BL, (c + 1) * BL)
        enc_c = enc[bs]                                   # [BL, S, H]
        ids = tgt[bs].T.reshape(R).astype(np.int32)       # r = t*BL + b
        h0T = np.ascontiguousarray(
            h0[bs].reshape(BL, 4, 128).transpose(2, 1, 0).reshape(128, 16)
        )
        in_maps.append({
            "emb": emb,
            "ids": np.ascontiguousarray(ids.reshape(2, 128, 1)),
            "h0T": h0T,
            "encT": np.ascontiguousarray(
                enc_c.transpose(2, 0, 1).reshape(H, BL * S)
            ),
            "encS": np.ascontiguousarray(
                enc_c.transpose(1, 0, 2).reshape(S, BL * H)
            ),
            "maskb": np.ascontiguousarray(
                np.where(mask[bs], 0.0, NEG).astype(np.float32).reshape(1, BL * S)
            ),
            "actmT": np.ascontiguousarray(np.broadcast_to(
                np.tile(
                    (np.arange(T)[:, None] < fra[bs][None, :]).astype(np.float32),
                    (1, 4),
                ).reshape(1, T * 16),
                (128, T * 16),
            )),
            "wihT": wihT,
            "whhT": whhT,
            "brow": brow,
            "bhhn4": bhhn4,
            "sel4": sel4,
            "wccT": wccT,
            "bcc": bcc4,
            "wo8": wo8i,
            "onesd": np.ones((1, 256), np.float32),
        })
    return in_maps


def assemble_output(core_outs):
    """core_outs: list of 8 arrays [R, V] bf16 (rows r = t*BL + b)."""
    out = np.empty((B, T, V), np.float32)
    for c in range(NC):
        o = np.asarray(core_outs[c]).astype(np.float32).reshape(T, BL, V)
        out[c * BL:(c + 1) * BL] = o.transpose(1, 0, 2)
    return out


def kernel(**inputs) -> np.ndarray:
    from concourse.bass_utils import run_bass_kernel_spmd
    nc = _get_program()
    in_maps = make_core_inputs(**inputs)
    res = run_bass_kernel_spmd(nc, in_maps, list(range(NC)))
    out = assemble_output([res.results[c]["out"] for c in range(NC)])
    b_out = np.asarray(inputs["b_out"], np.float32)
    if np.any(b_out):
        # Exact correction: log_softmax(l + b) = log_softmax(log_softmax(l) + b)
        x = out + b_out[None, None, :]
        m = x.max(axis=-1, keepdims=True)
        out = x - (m + np.log(np.exp(x - m).sum(axis=-1, keepdims=True)))
    return out
